# revision 4
# baseline (speedup 1.0000x reference)
"""WaveNet-style gated dilated conv layer on 8 Trainium2 NeuronCores.

Strategy: data-parallel over batch (B=8 -> 1 batch element per core).
Per core (batch b):
  z_tanh = sum_k Wc_tanh[k] @ x[:, t-d*(2-k)] + Wcond_tanh @ cond + bias
  z_sig  = likewise for the second half of the 2R conv channels
  h      = tanh(z_tanh) * sigmoid(z_sig)
  out    = W_out @ h, skip = W_skip @ h  (1x1 convs)

All matmuls run in bf16 with fp32 PSUM accumulation for z; the out/skip
1x1 matmuls write bf16 directly to PSUM (single-shot, no accumulation) so
the PSUM->SBUF copy and the output DMA both move half the bytes.  Outputs
are returned to the host as one packed bf16 tensor osk[R, 2, T] (out in
block 0, skip in block 1) and upcast to fp32 on host; b_out/b_skip are
zero per spec and added on host only if nonzero.

Schedule: time is cut into chunks (512..2048 cols); each chunk is
processed in 1024-col groups, each group as two 512-col PSUM halves
living in one 2-bank fp32 PSUM pair-tile.  Matmuls are ordered
weight-major (each loaded weight fires on both halves back-to-back) so
only 10 LDWEIGHTS are needed per 18 matmuls.  The out/skip matmuls for a
group are deferred to the start of the next group so the PE never waits
on the tanh/sigmoid activations or the gating multiply.

DMA rings: inputs + conv weights go on the SP HWDGE ring, outputs + the
small weights on the ACT HWDGE ring, so output triggers can fire as soon
as a chunk's copies land without head-of-line blocking the input stream.

TRN2 matmul instructions only have room for a single semaphore wait, so
input DMAs are "observed" by the PE via standalone ldweights instructions
at each chunk start before the first matmul that would otherwise combine
a DMA wait with a PSUM WAR wait.
"""

import sys

for _p in ("/opt/trn_rl_repo",):
    if _p not in sys.path:
        sys.path.append(_p)

from contextlib import ExitStack

import ml_dtypes
import numpy as np

import concourse.bacc as bacc
import concourse.bass as bass
import concourse.tile as tile
from concourse import mybir
from concourse.bass_utils import run_bass_kernel_spmd

B, CIN, T = 8, 128, 16384
R, S, CC, KW = 128, 128, 80, 3
NT = 512           # PSUM half width (one fp32 bank)
NG = 1024          # group width (one z pair-tile = two banks)
N_CORES = 8

BF16 = mybir.dt.bfloat16
FP32 = mybir.dt.float32
AF = mybir.ActivationFunctionType

_built = {}
_TRACE = False        # set True (e.g. by a test harness) to capture an NTFF profile
_last_results = None  # BassKernelResults of the most recent run


# Streaming chunk widths: small at the head (fast first-compute), large in
# the middle (few DMA triggers), small at the tail (fast final drain).
CHUNK_WIDTHS = [512, 1024, 1536, 2048, 2048, 2048, 2048, 2048, 1536, 1024, 512]
assert sum(CHUNK_WIDTHS) == T
CHUNK_STARTS = [sum(CHUNK_WIDTHS[:i]) for i in range(len(CHUNK_WIDTHS))]
NCH = len(CHUNK_WIDTHS)
PREFETCH = 2         # chunk lookahead


def _build(dilation: int) -> bass.Bass:
    pad = dilation * (KW - 1)

    nc = bacc.Bacc("TRN2", target_bir_lowering=False, debug=False, num_devices=N_CORES)

    x = nc.declare_dram_parameter("x", [CIN, pad + T], BF16, isOutput=False)
    cond = nc.declare_dram_parameter("cond", [CC, T], BF16, isOutput=False)
    # packed lhsT weights (already transposed to [Cin, Cout] on host)
    wconv = nc.declare_dram_parameter("wconv", [CIN, 2 * KW * R], BF16, isOutput=False)
    # w2: cols 0:R wcond_tan (rows 0:CC real), R:2R wcond_sig, 2R:3R wout, 3R:4R wskip
    w2 = nc.declare_dram_parameter("w2", [CIN, 4 * R], BF16, isOutput=False)
    zbias = nc.declare_dram_parameter("zbias", [R, 2], FP32, isOutput=False)

    osk = nc.declare_dram_parameter("osk", [R, 2, T], BF16, isOutput=True)

    with tile.TileContext(nc) as tc, ExitStack() as ctx:
        consts = ctx.enter_context(tc.tile_pool(name="consts", bufs=1))
        inpool = ctx.enter_context(tc.tile_pool(name="inp", bufs=PREFETCH + 2))
        hpool = ctx.enter_context(tc.tile_pool(name="h", bufs=2))
        opool = ctx.enter_context(tc.tile_pool(name="o", bufs=2))
        zpsum = ctx.enter_context(tc.tile_pool(name="zpsum", bufs=1, space="PSUM"))
        opsum = ctx.enter_context(tc.tile_pool(name="opsum", bufs=2, space="PSUM"))

        xc_tiles = [None] * NCH
        cc_tiles = [None] * NCH
        osb_tiles = [None] * NCH

        def load_chunk(g, cond_on_scalar=False):
            gs, gw = CHUNK_STARTS[g], CHUNK_WIDTHS[g]
            xc = inpool.tile([CIN, pad + gw], BF16, tag="xc")
            nc.sync.dma_start(xc[:], x[:, gs : gs + pad + gw])
            cc = inpool.tile([CC, gw], BF16, tag="cc")
            if cond_on_scalar:
                nc.scalar.dma_start(cc[:], cond[:, gs : gs + gw])
            else:
                nc.sync.dma_start(cc[:], cond[:, gs : gs + gw])
            xc_tiles[g], cc_tiles[g] = xc, cc

        # conv weights first on the SP ring (they gate the first real matmul),
        # then chunk 0's x; chunk 0's cond + small weights go on the ACT ring
        # so the two trigger streams run in parallel.
        wconv_sb = consts.tile([CIN, 2 * KW * R], BF16)
        nc.sync.dma_start(wconv_sb[:], wconv[:])
        load_chunk(0, cond_on_scalar=True)
        w2_sb = consts.tile([CIN, 4 * R], BF16)
        nc.scalar.dma_start(w2_sb[:], w2[:])
        zbias_sb = consts.tile([R, 2], FP32)
        nc.scalar.dma_start(zbias_sb[:], zbias[:])
        load_chunk(1)

        # Warm-up: short 128-col matmuls on memset SBUF start the PE HAM
        # ramp while the first chunk loads; two 1-column activations trigger
        # the tanh/sigmoid table load (~2.7us) on the ACT queue.
        garbage = consts.tile([CIN, R], BF16)
        act_sink = consts.tile([R, 1], FP32)
        nc.gpsimd.memset(garbage[:], 0.0)
        nc.gpsimd.memset(act_sink[:], 0.0)
        wz = zpsum.tile([R, NT], FP32, tag="ztp")
        for _ in range(10):
            nc.tensor.matmul(wz[:, 0:R], garbage[:], garbage[:], start=True, stop=True)
        nc.scalar.activation(act_sink[:], act_sink[:], AF.Tanh, bias=0.0)
        nc.scalar.activation(act_sink[:], act_sink[:], AF.Sigmoid, bias=0.0)

        # flat list of (chunk, in-chunk offset, group width, first, last)
        groups = []
        for c in range(NCH):
            gw = CHUNK_WIDTHS[c]
            for g0 in range(0, gw, NG):
                W = min(NG, gw - g0)
                groups.append((c, g0, W, g0 == 0, g0 + W == gw))

        prev = None  # (h, W, osb, o, chunk, was_chunk_last)

        def emit_prev():
            nonlocal prev
            if prev is None:
                return
            h, W, osb, o, c, last = prev
            prev = None
            for so in range(0, W, NT):
                po = opsum.tile([R, NT], FP32, tag="po", name="po")
                nc.tensor.matmul(
                    po[:], w2_sb[:, 2 * R : 3 * R], h[:, so : so + NT],
                    start=True, stop=True,
                )
                ps = opsum.tile([S, NT], FP32, tag="ps", name="ps")
                nc.tensor.matmul(
                    ps[:], w2_sb[:, 3 * R : 4 * R], h[:, so : so + NT],
                    start=True, stop=True,
                )
                nc.vector.tensor_copy(osb[:, 0, o + so : o + so + NT], po[:])
                nc.vector.tensor_copy(osb[:, 1, o + so : o + so + NT], ps[:])
            if last:
                gs, gw = CHUNK_STARTS[c], CHUNK_WIDTHS[c]
                nc.scalar.dma_start(osk[:, :, gs : gs + gw], osb[:, :, 0:gw])

        for c, g0, W, first, last in groups:
            if first:
                for g in range(c + 1, min(c + PREFETCH + 1, NCH)):
                    if xc_tiles[g] is None:
                        load_chunk(g)
                xc, cc = xc_tiles[c], cc_tiles[c]
                # let PE observe the chunk DMA sems on standalone ldweights
                nc.tensor.ldweights(xc[:, 0:R])
                nc.tensor.ldweights(cc[:, 0:R])
                osb_tiles[c] = opool.tile(
                    [R, 2, CHUNK_WIDTHS[c]], BF16, tag="osb", name="osb"
                )
            xc, cc = xc_tiles[c], cc_tiles[c]
            osb = osb_tiles[c]

            halves = [(g0, NT)] if W == NT else [(g0, NT), (g0 + NT, NT)]

            emit_prev()

            ztp = zpsum.tile([R, W], FP32, tag="ztp")
            zsp = zpsum.tile([R, W], FP32, tag="zsp")
            for half, (zp, base) in (("t", (ztp, 0)), ("s", (zsp, KW * R))):
                for k in range(KW):
                    wk = wconv_sb[:, base + k * R : base + (k + 1) * R]
                    for o, hw in halves:
                        nc.tensor.matmul(
                            zp[:, o - g0 : o - g0 + hw],
                            wk,
                            xc[:, o + dilation * k : o + dilation * k + hw],
                            start=(k == 0),
                            stop=False,
                        )
                wc = w2_sb[0:CC, (0 if half == "t" else R) : (R if half == "t" else 2 * R)]
                for o, hw in halves:
                    nc.tensor.matmul(
                        zp[:, o - g0 : o - g0 + hw],
                        wc,
                        cc[:, o : o + hw],
                        start=False,
                        stop=True,
                    )

            th = hpool.tile([R, W], BF16, tag="th")
            nc.scalar.activation(th[:], ztp[:], AF.Tanh, bias=zbias_sb[:, 0:1])
            sg = hpool.tile([R, W], BF16, tag="sg")
            nc.scalar.activation(sg[:], zsp[:], AF.Sigmoid, bias=zbias_sb[:, 1:2])
            h = hpool.tile([R, W], BF16, tag="h")
            nc.vector.tensor_mul(h[:], th[:], sg[:])

            prev = (h, W, osb, g0, c, last)

        emit_prev()

    nc.compile()
    return nc


def _pack_weights(w_conv, w_cond, w_out, w_skip, b_conv, b_cond):
    bf = ml_dtypes.bfloat16
    wconv_p = np.empty((CIN, 2 * KW * R), dtype=bf)
    for k in range(KW):
        wconv_p[:, k * R : (k + 1) * R] = w_conv[0:R, :, k].T.astype(bf)
        wconv_p[:, (KW + k) * R : (KW + k + 1) * R] = w_conv[R : 2 * R, :, k].T.astype(bf)
    w2_p = np.zeros((CIN, 4 * R), dtype=bf)
    w2_p[0:CC, 0:R] = w_cond[0:R, :, 0].T.astype(bf)
    w2_p[0:CC, R : 2 * R] = w_cond[R : 2 * R, :, 0].T.astype(bf)
    w2_p[:, 2 * R : 3 * R] = w_out[:, :, 0].T.astype(bf)
    w2_p[:, 3 * R : 4 * R] = w_skip[:, :, 0].T.astype(bf)
    zbias_p = np.stack(
        [b_conv[:R] + b_cond[:R], b_conv[R:] + b_cond[R:]], axis=1
    ).astype(np.float32)
    return wconv_p, w2_p, zbias_p


def kernel(**inputs):
    x = np.asarray(inputs["x"], dtype=np.float32)
    cond = np.asarray(inputs["cond"], dtype=np.float32)
    w_conv = np.asarray(inputs["w_conv"], dtype=np.float32)
    b_conv = np.asarray(inputs["b_conv"], dtype=np.float32)
    w_cond = np.asarray(inputs["w_cond"], dtype=np.float32)
    b_cond = np.asarray(inputs["b_cond"], dtype=np.float32)
    w_out = np.asarray(inputs["w_out"], dtype=np.float32)
    b_out = np.asarray(inputs["b_out"], dtype=np.float32)
    w_skip = np.asarray(inputs["w_skip"], dtype=np.float32)
    b_skip = np.asarray(inputs["b_skip"], dtype=np.float32)
    dilation = int(np.asarray(inputs["dilation"]))
    pad = dilation * (KW - 1)

    if dilation not in _built:
        _built[dilation] = _build(dilation)
    nc = _built[dilation]

    wconv_p, w2_p, zbias_p = _pack_weights(
        w_conv, w_cond, w_out, w_skip, b_conv, b_cond
    )
    bf = ml_dtypes.bfloat16
    xb = np.zeros((B, CIN, pad + T), dtype=bf)
    xb[:, :, pad:] = x.astype(bf)
    cb = np.ascontiguousarray(cond.astype(bf))

    in_maps = [
        {
            "x": xb[b],
            "cond": cb[b],
            "wconv": wconv_p,
            "w2": w2_p,
            "zbias": zbias_p,
        }
        for b in range(B)
    ]
    br = run_bass_kernel_spmd(nc, in_maps, list(range(N_CORES)), trace=_TRACE)
    global _last_results
    _last_results = br
    res = br.results
    osk = np.stack([np.asarray(res[b]["osk"]) for b in range(B)])
    output = osk[:, :, 0, :].astype(np.float32)
    skip = osk[:, :, 1, :].astype(np.float32)
    if b_out.any():
        output = output + b_out[None, :, None]
    if b_skip.any():
        skip = skip + b_skip[None, :, None]
    return (output, skip)
